# revision 1
# baseline (speedup 1.0000x reference)
"""Trainium2 Bass kernel for a batched GAT layer (BGATLayer).

Reference computation (per batch b of B=16, N=1024 nodes, F=512 features):
    h   = x @ W                                   # [N, F]
    s1  = h @ a1 ; s2 = h @ a2                    # [N]
    e   = leakyrelu(s1[:,None] + s2[None,:], 0.2) # [N, N]
    att = softmax(e, axis=1)                      # row softmax
    out = elu(att @ h + beta * h)                 # [N, F]

Sharding: batch B=16 split across 8 NeuronCores (2 batches/core, data
parallel); W/a/beta replicated.

Kernel structure, per batch (~126 us/core measured, f32r matmul path):
  * x is transposed 128x128-blockwise on the TensorEngine into xT (lhsT for
    h = x @ W; fp32 DMA transpose does not exist on trn2).
  * h = x @ W via f32r matmuls (fp32 bits in SBUF, reduced-precision PE mode,
    4x the strict-fp32 rate, measured end-to-end rel err ~3e-4).
    s1/s2 = x @ (W@a1, W@a2) come out as ROWS [2, N] from narrow-stationary
    matmuls (lhsT = w12 [128,2]) over xT.
  * e-rows: z[j,i] = s2[j] + s1[i] is a rank-2 outer product -> computed on
    the PE as a K=2 matmul (lhsT = [s2_row; ones], rhs = [ones; s1_row]),
    directly in the TRANSPOSED layout uT needs.  No broadcasts, no gpsimd
    (gpsimd elementwise measured ~20x slower than DVE).
  * softmax without max-subtraction (|e| <= ~25 is safe in fp32):
    uT[j] = exp(leakyrelu(z)) via ACT Prelu(alpha=0.2) -> SBUF -> ACT Exp
    (both live in the exp_and_others table -> no table switches; writing
    the lrelu to SBUF frees the PSUM bank after one op, which would
    otherwise pace the next z matmuls), alternating with a DVE
    tensor_scalar+scalar_tensor_tensor form to balance engines.  The NxN
    matrix is never transposed.
  * rowsum(u) via ones-stationary matmuls: rs = onesT @ uT accumulated over
    j -> a [1, N] row; 1/rs roundtrips through a DRAM scratch to become
    per-partition columns (a [1, N] DVE op would run on one lane at ~6.5us).
  * p = u @ h (f32r), epilogue: v = p*recip + beta*h (beta baked from the
    host-read input value), elu(v) = max(exp(min(v,0))-1, v) via
    DVE min -> ACT Exp -> DVE scalar_tensor_tensor.
  * the two batches are software-pipelined: batch-1 x loads/transposes/h
    overlap batch-0 attention; batch-1 uT tiles build during batch-0's
    second matmul.  Batch-0 x DMAs are issued before the weight DMAs so
    the PE starts as soon as the ~9us instruction-fetch startup ends.
"""

import sys

sys.path.insert(0, "/opt/trn_rl_repo")

from contextlib import ExitStack

import numpy as np

import concourse.bacc as bacc
import concourse.bass as bass
import concourse.mybir as mybir
from concourse.bass_utils import run_bass_kernel_spmd
from concourse.masks import make_identity
from concourse.tile import TileContext

P = 128
N_NODES = 1024
F = 512
B_TOTAL = 16
N_CORES = 8
B_PER_CORE = B_TOTAL // N_CORES
NK = F // P  # 4 contraction chunks for x @ W
NN = N_NODES // P  # 8 node chunks
ALPHA = 0.2

F32 = mybir.dt.float32
F32R = mybir.dt.float32r
AL = mybir.AluOpType
AF = mybir.ActivationFunctionType


def _r(ap):
    """float32r view of an fp32 AP (PE reduced-precision matmul mode)."""
    return ap.bitcast(F32R)


def build_nc(mm_fp32: bool = False, beta_val: float = 1.0) -> bass.Bass:
    cast = (lambda ap: ap) if mm_fp32 else _r

    nc = bacc.Bacc("TRN2")
    x_d = nc.dram_tensor("x", [B_PER_CORE, N_NODES, F], F32, kind="ExternalInput")
    w_d = nc.dram_tensor("W", [F, F], F32, kind="ExternalInput")
    a_d = nc.dram_tensor("a", [2 * F, 1], F32, kind="ExternalInput")
    beta_d = nc.dram_tensor("beta", [1], F32, kind="ExternalInput")
    out_d = nc.dram_tensor("out", [B_PER_CORE, N_NODES, F], F32, kind="ExternalOutput")
    # scratch for the reciprocal-rowsum row->column roundtrip
    r_d = nc.dram_tensor("r_scratch", [B_PER_CORE, N_NODES], F32)

    with TileContext(nc) as tc, ExitStack() as ctx:
        # ---------------- pools ----------------
        singles = ctx.enter_context(tc.tile_pool(name="singles", bufs=1))
        xin = ctx.enter_context(tc.tile_pool(name="xin", bufs=8))
        xtp = ctx.enter_context(tc.tile_pool(name="xtp", bufs=2))  # xT 16KB/part
        hpool = ctx.enter_context(tc.tile_pool(name="hpool", bufs=16))
        spool = ctx.enter_context(tc.tile_pool(name="spool", bufs=1))
        utp = ctx.enter_context(tc.tile_pool(name="utp", bufs=16))
        cpool = ctx.enter_context(tc.tile_pool(name="cpool", bufs=2))
        epool = ctx.enter_context(tc.tile_pool(name="epool", bufs=2))
        # PSUM: PS1 2x[128,1024](4 banks) PS2 2x[128,512](2) PS3 1x[2,1024](2)
        ps1 = ctx.enter_context(tc.tile_pool(name="ps1", bufs=2, space="PSUM"))
        ps2 = ctx.enter_context(tc.tile_pool(name="ps2", bufs=2, space="PSUM"))
        ps3 = ctx.enter_context(tc.tile_pool(name="ps3", bufs=1, space="PSUM"))

        # ---------------- prologue ----------------
        # float32r matmul operands must be *written* as f32r; gpsimd
        # memset/affine_select can't emit f32r, so constants go fp32 -> ACT.
        identf = singles.tile([P, P], F32, tag="identf")
        make_identity(nc, identf)
        ident = singles.tile([P, P], F32, tag="ident")
        nc.scalar.copy(out=cast(ident), in_=identf)

        onesf = singles.tile([P, 2], F32, tag="onesf")
        nc.gpsimd.memset(onesf, 1.0)
        ones2 = singles.tile([P, 2], F32, tag="ones2")
        nc.scalar.copy(out=cast(ones2), in_=onesf)
        onesrowf = singles.tile([1, N_NODES], F32, tag="onesrowf")
        nc.gpsimd.memset(onesrowf, 1.0)

        # weight-side tiles; their DMAs are emitted by load_weights() AFTER
        # the batch-0 x loads so the x tiles win the DMA queue
        a_flat = a_d.rearrange("f one -> (f one)")
        a1b = singles.tile([P, F], F32, tag="a1b")
        a2b = singles.tile([P, F], F32, tag="a2b")
        beta_sb = singles.tile([1, 1], F32, tag="beta_sb")
        w_sb = []
        for k in range(NK):
            wk = singles.tile([P, F], F32, tag=f"w_sb{k}")
            w_sb.append(wk)
        w12 = singles.tile([P, 2 * NK], F32, tag="w12")
        # z-matmul operands: zl = [s2_row; ones], zr = [ones; s1_row]
        zl = singles.tile([2, N_NODES], F32, tag="zl")
        zr = singles.tile([2, N_NODES], F32, tag="zr")

        def load_weights():
            nc.sync.dma_start(out=a1b, in_=a_flat[0:F].partition_broadcast(P))
            nc.sync.dma_start(out=a2b, in_=a_flat[F : 2 * F].partition_broadcast(P))
            # beta lands in SBUF only to keep the input bound (value baked)
            nc.sync.dma_start(out=beta_sb, in_=beta_d[0:1].unsqueeze(0))
            for k in range(NK):
                wk = w_sb[k]
                nc.sync.dma_start(out=cast(wk), in_=cast(w_d[k * P : (k + 1) * P, :]))
                w12f = cpool.tile([P, 2], F32, tag="w12f")
                prod = cpool.tile([P, F], F32, tag="wa_prod")
                for j, ab in enumerate((a1b, a2b)):
                    nc.vector.tensor_tensor(
                        out=prod, in0=wk.bitcast(F32), in1=ab, op=AL.mult
                    )
                    nc.vector.reduce_sum(
                        out=w12f[:, j : j + 1], in_=prod, axis=mybir.AxisListType.X
                    )
                nc.scalar.copy(out=cast(w12[:, 2 * k : 2 * k + 2]), in_=w12f)
            # compute engines can't address partition offset 1 -> row writes
            # go through DMA (any-partition capable)
            nc.sync.dma_start(out=cast(zl[1:2, :]), in_=cast(onesrowf))
            nc.sync.dma_start(out=cast(zr[0:1, :]), in_=cast(onesrowf))

        # ---------------- PE warm-up ----------------
        # the HAM clock gate keeps a cold PE at 1.2 GHz; ~40 dummy transposes
        # during the initial DMA window hold the activity monitor busy so real
        # matmuls start at 2.4 GHz
        for _ in range(6):
            wp = ps1.tile([P, N_NODES], F32, tag="ps1")
            nc.tensor.transpose(cast(wp[:, 0:P]), cast(ident), cast(ident))
            nc.tensor.transpose(cast(wp[:, P : 2 * P]), cast(ident), cast(ident))

        # ---------------- per-batch phases ----------------
        xt_alls = {}
        h_sbs = {}
        uts = {}
        rcols = {}

        x_tiles = {}

        def phase_A_dma(b):  # issue all x loads for this batch
            x_tiles[b] = []
            for n in range(NN):
                x_t = xin.tile([P, F], F32, tag="x_t")
                nc.sync.dma_start(
                    out=cast(x_t), in_=cast(x_d[b, n * P : (n + 1) * P, :])
                )
                x_tiles[b].append(x_t)

        def emit_A_tile(b, n):
            x_t = x_tiles[b][n]
            xt_all = xt_alls[b]
            xp = ps1.tile([P, N_NODES], F32, tag="ps1")
            for k in range(NK):
                nc.tensor.transpose(
                    cast(xp[:, k * P : (k + 1) * P]),
                    cast(x_t[:, k * P : (k + 1) * P]),
                    cast(ident),
                )
            dst = xt_all.rearrange("p (k c) -> p k c", k=NK)[
                :, :, n * P : (n + 1) * P
            ]
            src = xp[:, 0:F].rearrange("p (k c) -> p k c", k=NK)
            nc.vector.tensor_copy(out=cast(dst), in_=cast(src))

        def phase_A(b):  # transpose into xT
            xt_all = xtp.tile([P, NK * N_NODES], F32, tag="xt_all")
            xt_alls[b] = xt_all
            for n in range(NN):
                emit_A_tile(b, n)

        def phase_S(b):  # s rows -> zl/zr operands
            xt_all = xt_alls[b]
            s_ps = ps3.tile([2, N_NODES], F32, tag="ps3")
            for k in range(NK):
                for hh in range(2):
                    nc.tensor.matmul(
                        s_ps[:, hh * F : (hh + 1) * F],
                        lhsT=cast(w12[:, 2 * k : 2 * k + 2]),
                        rhs=cast(
                            xt_all[:, k * N_NODES + hh * F : k * N_NODES + (hh + 1) * F]
                        ),
                        start=(k == 0),
                        stop=(k == NK - 1),
                    )
            s_sb = spool.tile([2, N_NODES], F32, tag="s_sb")
            nc.vector.tensor_copy(out=s_sb, in_=s_ps)
            nc.sync.dma_start(out=cast(zl[0:1, :]), in_=cast(s_sb[1:2, :]))  # s2
            nc.sync.dma_start(out=cast(zr[1:2, :]), in_=cast(s_sb[0:1, :]))  # s1

        def emit_B_tile(b, n):
            xt_all = xt_alls[b]
            h_ps = ps2.tile([P, F], F32, tag="ps2")
            for k in range(NK):
                nc.tensor.matmul(
                    h_ps,
                    lhsT=cast(
                        xt_all[:, k * N_NODES + n * P : k * N_NODES + (n + 1) * P]
                    ),
                    rhs=cast(w_sb[k]),
                    start=(k == 0),
                    stop=(k == NK - 1),
                )
            ht = hpool.tile([P, F], F32, tag="h_sb")
            nc.scalar.copy(out=cast(ht), in_=h_ps)
            h_sbs[b].append(ht)

        def phase_B(b):  # h = x @ W
            h_sbs[b] = []
            for n in range(NN):
                emit_B_tile(b, n)

        def emit_C_tile(b, j, path="act"):
            # uT[j][p, i] = exp(lrelu(s2[j*128+p] + s1[i]))
            z_ps = ps1.tile([P, N_NODES], F32, tag="ps1")
            for hh in range(2):
                nc.tensor.matmul(
                    z_ps[:, hh * F : (hh + 1) * F],
                    lhsT=cast(zl[:, j * P : (j + 1) * P]),
                    rhs=cast(zr[:, hh * F : (hh + 1) * F]),
                    start=True,
                    stop=True,
                )
            # lrelu lands in SBUF (not in-place in PSUM) so the ps1 slot
            # frees after ONE op instead of being held through the exp --
            # the slot hold time paces the next z matmuls on the PE
            lr = cpool.tile([P, N_NODES], F32, tag="lr")
            if path == "act":
                # parametric_relu and exp share one ACT table set:
                # two ACT passes, zero DVE work
                nc.scalar.activation(out=lr, in_=z_ps, func=AF.Prelu, alpha=ALPHA)
            else:
                # DVE leaky-relu (balances ACT when it is the pacer):
                # t = 0.2z ; lr = max(t, z)
                t = cpool.tile([P, N_NODES], F32, tag="wa_prod")
                nc.vector.tensor_scalar_mul(t, z_ps, ALPHA)
                nc.vector.scalar_tensor_tensor(
                    out=lr, in0=t, scalar=1.0, in1=z_ps, op0=AL.mult, op1=AL.max
                )
            u = utp.tile([P, N_NODES], F32, tag="ut")
            nc.scalar.activation(out=cast(u), in_=lr, func=AF.Exp)
            uts[b].append(u)

        def phase_C(b):
            uts[b] = []
            for j in range(NN):
                emit_C_tile(b, j)

        def phase_R(b):  # rowsum -> reciprocal columns
            ut = uts[b]
            rs_ps = ps3.tile([2, N_NODES], F32, tag="ps3")
            for j in range(NN):
                for hh in range(2):
                    nc.tensor.matmul(
                        rs_ps[:, hh * F : (hh + 1) * F],
                        lhsT=cast(ones2),
                        rhs=cast(ut[j][:, hh * F : (hh + 1) * F]),
                        start=(j == 0),
                        stop=(j == NN - 1),
                    )
            # rowsum row -> per-partition columns through DRAM; the
            # reciprocal runs on the [128, 8] column form (a [1, N] DVE op
            # would grind on a single partition lane at ~6.5us)
            rrow = spool.tile([1, N_NODES], F32, tag="rrow")
            nc.vector.tensor_copy(out=rrow, in_=rs_ps[0:1, :])
            nc.sync.dma_start(out=r_d[b].unsqueeze(0), in_=rrow)
            rcraw = spool.tile([P, NN], F32, tag="rcraw")
            nc.sync.dma_start(out=rcraw, in_=r_d[b].rearrange("(n p) -> p n", p=P))
            rcol = spool.tile([P, NN], F32, tag="rcol")
            rcols[b] = rcol
            nc.vector.reciprocal(out=rcol, in_=rcraw)

        def emit_DE_tile(b, n):  # p[n] = u @ h + fused ELU epilogue
            ut, h_sb, rcol = uts[b], h_sbs[b], rcols[b]
            if True:
                p_ps = ps2.tile([P, F], F32, tag="ps2")
                for j in range(NN):
                    nc.tensor.matmul(
                        p_ps,
                        lhsT=cast(ut[j][:, n * P : (n + 1) * P]),
                        rhs=cast(h_sb[j]),
                        start=(j == 0),
                        stop=(j == NN - 1),
                    )
                hin = h_sb[n].bitcast(F32)
                if beta_val == 1.0:
                    hb = hin
                else:
                    hb = epool.tile([P, F], F32, tag="hb")
                    nc.vector.tensor_scalar_mul(hb, hin, float(beta_val))
                v = epool.tile([P, F], F32, tag="v")
                # v = p * (1/rowsum) + beta*h
                nc.vector.scalar_tensor_tensor(
                    out=v, in0=p_ps, scalar=rcol[:, n : n + 1], in1=hb,
                    op0=AL.mult, op1=AL.add,
                )
                m = epool.tile([P, F], F32, tag="m")
                if b == 0:
                    nc.vector.tensor_scalar_min(m, v, 0.0)
                else:
                    # min(v,0) = -relu(-v); ACT is idle during the tail
                    nc.scalar.activation(out=m, in_=v, func=AF.Relu, scale=-1.0)
                em = epool.tile([P, F], F32, tag="em")
                nc.scalar.activation(
                    out=em, in_=m, func=AF.Exp, scale=(1.0 if b == 0 else -1.0)
                )
                o = epool.tile([P, F], F32, tag="m")
                # elu(v) = max(exp(min(v,0)) - 1, v)
                nc.vector.scalar_tensor_tensor(
                    out=o, in0=em, scalar=-1.0, in1=v, op0=AL.add, op1=AL.max
                )
                nc.sync.dma_start(out=out_d[b, n * P : (n + 1) * P, :], in_=o)

        # software-pipelined emission.  batch-0 x loads were issued before
        # the prologue DMAs (same queue) so the PE can start immediately;
        # C phases interleave with matmul phases so ACT never paces the PE.
        phase_A_dma(0)
        load_weights()
        phase_A(0)
        phase_S(0)
        phase_A_dma(1)
        uts[0] = []
        h_sbs[0] = []
        for i in range(NN):
            emit_B_tile(0, i)
        for i in range(NN):
            emit_C_tile(0, i, path="act" if i % 2 == 0 else "dve")
        phase_A(1)
        phase_S(1)
        phase_R(0)
        phase_B(1)
        uts[1] = []
        for j in range(3):
            emit_C_tile(1, j)
        for i in range(NN):
            if 3 + i < NN:
                emit_C_tile(1, 3 + i)
            if i == 5:
                phase_R(1)
            emit_DE_tile(0, i)
        for i in range(NN):
            emit_DE_tile(1, i)

    nc.finalize()
    return nc


_NC_CACHE = {}


def _get_nc(mm_fp32: bool, beta_val: float) -> bass.Bass:
    key = (bool(mm_fp32), float(beta_val))
    if key not in _NC_CACHE:
        _NC_CACHE[key] = build_nc(mm_fp32=key[0], beta_val=key[1])
    return _NC_CACHE[key]


def kernel(x, W, a, beta, _trace=False, _mm_fp32=False):
    x = np.ascontiguousarray(x, dtype=np.float32)
    W = np.ascontiguousarray(W, dtype=np.float32)
    a = np.ascontiguousarray(a, dtype=np.float32)
    beta = np.ascontiguousarray(beta, dtype=np.float32)

    nc = _get_nc(_mm_fp32, float(beta.reshape(-1)[0]))
    in_maps = [
        {
            "x": x[c * B_PER_CORE : (c + 1) * B_PER_CORE],
            "W": W,
            "a": a,
            "beta": beta,
        }
        for c in range(N_CORES)
    ]
    res = run_bass_kernel_spmd(nc, in_maps, core_ids=list(range(N_CORES)), trace=_trace)
    out = np.concatenate([r["out"] for r in res.results], axis=0)
    if _trace:
        kernel.last_exec_time_ns = res.exec_time_ns
        kernel.last_results = res
    return out


if __name__ == "__main__":
    rng = np.random.default_rng(0)
    x = rng.standard_normal((B_TOTAL, N_NODES, F), dtype=np.float32)
    W = rng.standard_normal((F, F), dtype=np.float32) * 0.05
    a = rng.standard_normal((2 * F, 1), dtype=np.float32) * 0.05
    beta = np.ones((1,), dtype=np.float32)
    out = kernel(x, W, a, beta)
    print("out", out.shape, out.dtype)



# revision 8
# speedup vs baseline: 1.2466x; 1.2466x over previous
"""Trainium2 Bass kernel for a batched GAT layer (BGATLayer).

Reference computation (per batch b of B=16, N=1024 nodes, F=512 features):
    h   = x @ W                                   # [N, F]
    s1  = h @ a1 ; s2 = h @ a2                    # [N]
    e   = leakyrelu(s1[:,None] + s2[None,:], 0.2) # [N, N]
    att = softmax(e, axis=1)                      # row softmax
    out = elu(att @ h + beta * h)                 # [N, F]

Sharding: batch B=16 split across 8 NeuronCores (2 batches/core, data
parallel); W/a/beta replicated.

v2 design (v1 measured 147us; PE busy 107us incl ~40us at HAM half
clock from C-phase starvation, plus a 19us epilogue tail):
  * All matmul operands are bf16 (tolerance is 2e-2; bf16 adds ~4e-3).
    Streaming rate is the same 1 cyc/row as f32r@512, but LDWEIGHTS is
    ~2x faster and SBUF footprint halves.
  * uT tiles are computed with ZERO PE work: uT[j][p,i] =
    exp(prelu(s1[i] + s2[j*128+p])).  s1 enters as a [128, N]
    partition-broadcast tile (SBUF->SBUF DMA), s2 as a per-partition
    bias column (ACT activation bias= accepts a [128,1] AP; DVE path
    uses tensor_scalar with an AP scalar).  v1 computed each z tile as
    a K=2 PE matmul that ping-ponged with ACT and starved the PE into
    the HAM's k=4/8 duty-cycle downclock.
  * PE stream is matmul-only and back-to-back:
      warmup T0 S0 B0 T1 S1 B1 R0 DE0(p) R1 DE1(p)
    so the activity monitor keeps the clock at max.
  * rowsum still via ones-stationary matmuls (cheap: 2-row stationary),
    reciprocal through the DRAM row->column roundtrip.
  * epilogue per tile: v = p*recip + h (DVE stt, mixed f32/bf16),
    m = min(v,0) (DVE), em = exp(m) (ACT), out = max(em-1, v) (DVE),
    trailing the p matmuls tile-by-tile instead of bunching at the end.
"""

import sys

sys.path.insert(0, "/opt/trn_rl_repo")

from contextlib import ExitStack

import numpy as np

import concourse.bacc as bacc
import concourse.bass as bass
import concourse.mybir as mybir
from concourse.bass_utils import run_bass_kernel_spmd
from concourse.masks import make_identity
from concourse.tile import TileContext

P = 128
N_NODES = 1024
F = 512
B_TOTAL = 16
N_CORES = 8
B_PER_CORE = B_TOTAL // N_CORES
NK = F // P  # 4 contraction chunks for x @ W
NN = N_NODES // P  # 8 node chunks
ALPHA = 0.2

F32 = mybir.dt.float32
F32R = mybir.dt.float32r
BF16 = mybir.dt.bfloat16
AL = mybir.AluOpType
AF = mybir.ActivationFunctionType

# which C-phase tiles run prelu on ACT (True) vs DVE (False)
C_ON_ACT = [True, True, False, True, True, False, True, True]
# which h-tile PSUM->SBUF copies go on ACT (True) vs DVE (False)
H_ON_ACT = [True, False, True, False, True, False, True, False]


def _r(ap):
    """float32r view of an fp32 AP (PE reduced-precision matmul mode)."""
    return ap.bitcast(F32R)


def build_nc(beta_val: float = 1.0) -> bass.Bass:
    nc = bacc.Bacc("TRN2")
    x_d = nc.dram_tensor("x", [B_PER_CORE, N_NODES, F], F32, kind="ExternalInput")
    w_d = nc.dram_tensor("W", [F, F], F32, kind="ExternalInput")
    a_d = nc.dram_tensor("a", [2 * F, 1], F32, kind="ExternalInput")
    beta_d = nc.dram_tensor("beta", [1], F32, kind="ExternalInput")
    out_d = nc.dram_tensor("out", [B_PER_CORE, N_NODES, F], F32, kind="ExternalOutput")
    # scratch for row->per-partition-column roundtrips
    r_d = nc.dram_tensor("r_scratch", [B_PER_CORE, N_NODES], F32)
    s_d = nc.dram_tensor("s_scratch", [B_PER_CORE, 2, N_NODES], F32)

    with TileContext(nc) as tc, ExitStack() as ctx:
        # ---------------- pools ----------------
        singles = ctx.enter_context(tc.tile_pool(name="singles", bufs=1))
        xin = ctx.enter_context(tc.tile_pool(name="xin", bufs=16))
        xtp = ctx.enter_context(tc.tile_pool(name="xtp", bufs=2))  # xT bf16
        hpool = ctx.enter_context(tc.tile_pool(name="hpool", bufs=16))
        spool = ctx.enter_context(tc.tile_pool(name="spool", bufs=2))
        utp = ctx.enter_context(tc.tile_pool(name="utp", bufs=16))
        lrp = ctx.enter_context(tc.tile_pool(name="lrp", bufs=4))
        qp = ctx.enter_context(tc.tile_pool(name="qp", bufs=2))
        epool = ctx.enter_context(tc.tile_pool(name="epool", bufs=3))
        # PSUM: ps_tr 2x[128,512](2 banks) ps_mm 4x[128,512](4) ps_s 1x[2,1024](2)
        ps_tr = ctx.enter_context(tc.tile_pool(name="ps_tr", bufs=2, space="PSUM"))
        ps_mm = ctx.enter_context(tc.tile_pool(name="ps_mm", bufs=4, space="PSUM"))
        ps_s = ctx.enter_context(tc.tile_pool(name="ps_s", bufs=1, space="PSUM"))

        # ---------------- prologue ----------------
        identf = singles.tile([P, P], F32, tag="identf")
        make_identity(nc, identf)
        ident = singles.tile([P, P], F32, tag="ident")
        nc.scalar.copy(out=_r(ident), in_=identf)

        ones2b = singles.tile([P, 2], BF16, tag="ones2b")
        nc.gpsimd.memset(ones2b, 1.0)

        a_flat = a_d.rearrange("f one -> (f one)")
        a1b = singles.tile([P, F], F32, tag="a1b")
        a2b = singles.tile([P, F], F32, tag="a2b")
        beta_sb = singles.tile([1, 1], F32, tag="beta_sb")
        w_sb = []
        wb = []
        for k in range(NK):
            wk = singles.tile([P, F], F32, tag=f"w_sb{k}")
            w_sb.append(wk)
            wbk = singles.tile([P, F], BF16, tag=f"wb{k}")
            wb.append(wbk)
        w12b = singles.tile([P, 2 * NK], BF16, tag="w12b")

        def load_weights():
            nc.sync.dma_start(out=a1b, in_=a_flat[0:F].partition_broadcast(P))
            nc.sync.dma_start(out=a2b, in_=a_flat[F : 2 * F].partition_broadcast(P))
            # beta lands in SBUF only to keep the input bound (value baked)
            nc.sync.dma_start(out=beta_sb, in_=beta_d[0:1].unsqueeze(0))
            for k in range(NK):
                nc.sync.dma_start(out=w_sb[k], in_=w_d[k * P : (k + 1) * P, :])
                # bf16 copy of W for the h matmul (moving operand)
                nc.scalar.copy(out=wb[k], in_=w_sb[k])
                w12f = qp.tile([P, 2], F32, tag="w12f")
                prod = qp.tile([P, F], F32, tag="wa_prod")
                for j, ab in enumerate((a1b, a2b)):
                    nc.vector.tensor_tensor(
                        out=prod, in0=w_sb[k], in1=ab, op=AL.mult
                    )
                    nc.vector.reduce_sum(
                        out=w12f[:, j : j + 1], in_=prod, axis=mybir.AxisListType.X
                    )
                nc.scalar.copy(out=w12b[:, 2 * k : 2 * k + 2], in_=w12f)

        # ---------------- PE warm-up ----------------
        # hold the activity monitor busy during the initial DMA window so
        # real matmuls start at the max clock
        for _ in range(4):
            wp = ps_tr.tile([P, F], F32, tag="ps_tr")
            nc.tensor.transpose(_r(wp[:, 0:P]), _r(ident), _r(ident))
            nc.tensor.transpose(_r(wp[:, P : 2 * P]), _r(ident), _r(ident))

        # ---------------- per-batch state ----------------
        xt_alls = {}
        h_sbs = {}
        uts = {}
        rcols = {}
        s1bs = {}
        s2cols = {}
        x_tiles = {}

        def phase_A_dma(b):  # issue all x loads for this batch
            x_tiles[b] = []
            for n in range(NN):
                x_t = xin.tile([P, F], F32, tag="x_t")
                nc.sync.dma_start(out=_r(x_t), in_=_r(x_d[b, n * P : (n + 1) * P, :]))
                x_tiles[b].append(x_t)

        def phase_T(b):  # transpose x into bf16 xT
            xt_all = xtp.tile([P, NK * N_NODES], BF16, tag="xt_all")
            xt_alls[b] = xt_all
            for n in range(NN):
                x_t = x_tiles[b][n]
                xp = ps_tr.tile([P, F], F32, tag="ps_tr")
                for k in range(NK):
                    nc.tensor.transpose(
                        _r(xp[:, k * P : (k + 1) * P]),
                        _r(x_t[:, k * P : (k + 1) * P]),
                        _r(ident),
                    )
                dst = xt_all.rearrange("p (k c) -> p k c", k=NK)[
                    :, :, n * P : (n + 1) * P
                ]
                src = xp.rearrange("p (k c) -> p k c", k=NK)
                nc.vector.tensor_copy(out=dst, in_=src)

        def phase_S(b):  # s rows -> s2 bias columns + s1 broadcast row
            xt_all = xt_alls[b]
            s_ps = ps_s.tile([2, N_NODES], F32, tag="ps_s")
            for k in range(NK):
                for hh in range(2):
                    nc.tensor.matmul(
                        s_ps[:, hh * F : (hh + 1) * F],
                        lhsT=w12b[:, 2 * k : 2 * k + 2],
                        rhs=xt_all[:, k * N_NODES + hh * F : k * N_NODES + (hh + 1) * F],
                        start=(k == 0),
                        stop=(k == NK - 1),
                    )
            s_sb = spool.tile([2, N_NODES], F32, tag="s_sb")
            nc.vector.tensor_copy(out=s_sb, in_=s_ps)
            # both s rows to DRAM, then read back as broadcast / columns
            nc.sync.dma_start(out=s_d[b], in_=s_sb)
            s1b = spool.tile([P, N_NODES], F32, tag="s1b")
            s1bs[b] = s1b
            nc.sync.dma_start(out=s1b, in_=s_d[b, 0].partition_broadcast(P))
            s2col = spool.tile([P, NN], F32, tag="s2col")
            s2cols[b] = s2col
            nc.sync.dma_start(out=s2col, in_=s_d[b, 1].rearrange("(n p) -> p n", p=P))

        def emit_C_tile(b, j):
            # uT[j][p, i] = exp(prelu(s1[i] + s2[j*128+p])), no PE work
            s1b, s2col = s1bs[b], s2cols[b]
            s2j = s2col[:, j : j + 1]
            lr = lrp.tile([P, N_NODES], F32, tag="lr")
            if C_ON_ACT[j]:
                nc.scalar.activation(
                    out=lr, in_=s1b, func=AF.Prelu, bias=s2j, alpha=ALPHA
                )
            else:
                # DVE: q = alpha*(s1+s2); lr = max(s1+s2, q)
                q = qp.tile([P, N_NODES], F32, tag="q")
                nc.vector.tensor_scalar(
                    out=q, in0=s1b, scalar1=s2j, scalar2=ALPHA, op0=AL.add, op1=AL.mult
                )
                nc.vector.scalar_tensor_tensor(
                    out=lr, in0=s1b, scalar=s2j, in1=q, op0=AL.add, op1=AL.max
                )
            u = utp.tile([P, N_NODES], BF16, tag="ut")
            nc.scalar.activation(out=u, in_=lr, func=AF.Exp)
            uts[b][j] = u

        def phase_C(b):
            # ACT-prelu'd tiles first so the ACT queue never stalls waiting
            # for a DVE-produced lr; DVE tiles' lrs are ready by the time
            # the ACT queue reaches their Exp ops.
            uts[b] = [None] * NN
            for j in range(NN):
                if C_ON_ACT[j]:
                    emit_C_tile(b, j)
            for j in range(NN):
                if not C_ON_ACT[j]:
                    emit_C_tile(b, j)

        def phase_B(b):  # h = x @ W  (bf16 out for the p matmul + epilogue)
            xt_all = xt_alls[b]
            h_sbs[b] = []
            for n in range(NN):
                h_ps = ps_mm.tile([P, F], F32, tag="ps_mm")
                for k in range(NK):
                    nc.tensor.matmul(
                        h_ps,
                        lhsT=xt_all[:, k * N_NODES + n * P : k * N_NODES + (n + 1) * P],
                        rhs=wb[k],
                        start=(k == 0),
                        stop=(k == NK - 1),
                    )
                ht = hpool.tile([P, F], BF16, tag="h_sb")
                if H_ON_ACT[n]:
                    nc.scalar.copy(out=ht, in_=h_ps)
                else:
                    nc.vector.tensor_copy(out=ht, in_=h_ps)
                h_sbs[b].append(ht)

        def phase_R(b):  # rowsum -> reciprocal columns
            ut = uts[b]
            rs_ps = ps_s.tile([2, N_NODES], F32, tag="ps_s")
            for j in range(NN):
                for hh in range(2):
                    nc.tensor.matmul(
                        rs_ps[:, hh * F : (hh + 1) * F],
                        lhsT=ones2b,
                        rhs=ut[j][:, hh * F : (hh + 1) * F],
                        start=(j == 0),
                        stop=(j == NN - 1),
                    )
            rrow = spool.tile([1, N_NODES], F32, tag="rrow")
            nc.vector.tensor_copy(out=rrow, in_=rs_ps[0:1, :])
            nc.sync.dma_start(out=r_d[b].unsqueeze(0), in_=rrow)
            rcraw = spool.tile([P, NN], F32, tag="rcraw")
            nc.sync.dma_start(out=rcraw, in_=r_d[b].rearrange("(n p) -> p n", p=P))
            rcol = spool.tile([P, NN], F32, tag="rcol")
            rcols[b] = rcol
            nc.vector.reciprocal(out=rcol, in_=rcraw)

        def emit_DE_tile(b, n):  # p[n] = u @ h + fused ELU epilogue
            ut, h_sb, rcol = uts[b], h_sbs[b], rcols[b]
            p_ps = ps_mm.tile([P, F], F32, tag="ps_mm")
            for j in range(NN):
                nc.tensor.matmul(
                    p_ps,
                    lhsT=ut[j][:, n * P : (n + 1) * P],
                    rhs=h_sb[j],
                    start=(j == 0),
                    stop=(j == NN - 1),
                )
            hin = h_sb[n]
            if beta_val != 1.0:
                hb = epool.tile([P, F], F32, tag="hb")
                nc.vector.tensor_scalar_mul(hb, hin, float(beta_val))
                hin = hb
            v = epool.tile([P, F], F32, tag="v")
            # v = p * (1/rowsum) + beta*h
            nc.vector.scalar_tensor_tensor(
                out=v, in0=p_ps, scalar=rcol[:, n : n + 1], in1=hin,
                op0=AL.mult, op1=AL.add,
            )
            m = epool.tile([P, F], F32, tag="m")
            nc.vector.tensor_scalar_min(m, v, 0.0)
            em = epool.tile([P, F], F32, tag="em")
            nc.scalar.activation(out=em, in_=m, func=AF.Exp)
            o = epool.tile([P, F], F32, tag="o")
            # elu(v) = max(exp(min(v,0)) - 1, v)
            nc.vector.scalar_tensor_tensor(
                out=o, in0=em, scalar=-1.0, in1=v, op0=AL.add, op1=AL.max
            )
            nc.sync.dma_start(out=out_d[b, n * P : (n + 1) * P, :], in_=o)

        # ------------- software-pipelined emission -------------
        # PE order: warmup T0 S0 B0 T1 S1 B1 R0 DE0 R1 DE1 -- back-to-back
        # matmuls, never paced by ACT/DVE.  C phases are pure ACT/DVE and
        # run concurrently (C0 under B0/T1, C1 under B1/DE0).
        phase_A_dma(0)
        load_weights()
        phase_T(0)
        phase_S(0)
        phase_A_dma(1)
        phase_B(0)
        phase_T(1)
        phase_C(0)
        phase_S(1)
        phase_B(1)
        phase_C(1)
        phase_R(0)
        for n in range(5):
            emit_DE_tile(0, n)
        phase_R(1)
        for n in range(5, NN):
            emit_DE_tile(0, n)
        for n in range(NN):
            emit_DE_tile(1, n)

    nc.finalize()
    return nc


_NC_CACHE = {}


def _get_nc(beta_val: float) -> bass.Bass:
    key = float(beta_val)
    if key not in _NC_CACHE:
        _NC_CACHE[key] = build_nc(beta_val=key)
    return _NC_CACHE[key]


def kernel(x, W, a, beta, _trace=False, _mm_fp32=False):
    x = np.ascontiguousarray(x, dtype=np.float32)
    W = np.ascontiguousarray(W, dtype=np.float32)
    a = np.ascontiguousarray(a, dtype=np.float32)
    beta = np.ascontiguousarray(beta, dtype=np.float32)

    nc = _get_nc(float(beta.reshape(-1)[0]))
    in_maps = [
        {
            "x": x[c * B_PER_CORE : (c + 1) * B_PER_CORE],
            "W": W,
            "a": a,
            "beta": beta,
        }
        for c in range(N_CORES)
    ]
    res = run_bass_kernel_spmd(nc, in_maps, core_ids=list(range(N_CORES)), trace=_trace)
    out = np.concatenate([r["out"] for r in res.results], axis=0)
    if _trace:
        kernel.last_exec_time_ns = res.exec_time_ns
        kernel.last_results = res
    return out


if __name__ == "__main__":
    rng = np.random.default_rng(0)
    x = rng.standard_normal((B_TOTAL, N_NODES, F), dtype=np.float32)
    W = rng.standard_normal((F, F), dtype=np.float32) * 0.05
    a = rng.standard_normal((2 * F, 1), dtype=np.float32) * 0.05
    beta = np.ones((1,), dtype=np.float32)
    out = kernel(x, W, a, beta)
    print("out", out.shape, out.dtype)
